# revision 34
# baseline (speedup 1.0000x reference)
"""ChebGraphConv (K=3) Trainium2 kernel.

Reference computation (per batch b, time t; x0 = x[b,:,t,:].T in [N, C_IN]):
    out = x0 @ W0 + (G @ x0) @ W1 + (2 G (G @ x0) - x0) @ W2 + bias
Rewritten (G commutes with channel matmuls):
    P2  = x0 @ (2 W2)
    U   = G @ P2 + x0 @ W1
    out = G @ U  + x0 @ (W0 - W2) + bias

Sharding: batch B=16 split over 8 cores (2 b per core).  gso/weights
replicated.  All matmuls in bfloat16 (full PE rate at any output width;
fp32r moving operands drop to 1/4 rate below 256-wide).  PSUM
accumulation stays fp32; output stored bf16 as contiguous [128,512]
blocks (1KB DMA runs; host untangles the layout and upcasts — rel err
~4.3e-3 vs the 2e-2 gate; fp8/DoubleRow measured at 3-4.5e-2 and
rejected).  Steady state is PE-bound at ~267us busy (bf16 floor 259.5us
= 77824 rows/iter x 8 iters at 2.4GHz); DVE casts/adds and both DMA
queues have 2x+ slack; zero mid-stream PE gaps.  Measured: 287763 ns.

Per core loop over 8 blocks (b, blk) with 16 t's each:
  A[q]   = x[b, :, blk*16+4q : +4, :]  as [128=(c,d4), 1024=n]   (4 quads)
  S1: P2[nchunk 128, 512=(q,d,j)] = sum_c A[q]^T Wblk(2W2)        (block-diag)
  S2: U = G-matmuls (lhsT = gsoT chunks) + block-diag W1 accum
  S3: out = G-matmuls on U + block-diag (W0-W2) accum, + bias, DMA out
"""
import numpy as np

B, C_IN, T, N = 16, 32, 64, 1024
C_OUT = 32
N_CORES = 8
B_PER = B // N_CORES          # 2
N_BLK = T // 16               # 4 blocks of 16 t's per b -> 8 iters per core
NCH = N // 128                # 8 chunks of the node dim

_CACHE = {}


def _split_multi_waits(nc, mybir, max_waits: int = 1):
    """Walrus rejects instructions whose ISA struct can't hold all their sync
    waits (fp32 self-loading matmul: 1).  Hoist excess waits onto inserted
    same-engine NoOps, which execute in order before the instruction."""
    import copy

    protos = {}

    def make_nop(engine, name):
        if engine not in protos:
            eng_map = {
                mybir.EngineType.SP: nc.sync,
                mybir.EngineType.PE: nc.tensor,
                mybir.EngineType.DVE: nc.vector,
                mybir.EngineType.Activation: nc.scalar,
                mybir.EngineType.Pool: nc.gpsimd,
            }
            proto = eng_map[engine].nop().ins
            for f in nc.m.functions:
                for blk in f.blocks:
                    insts = list(blk.instructions)
                    if insts and insts[-1] is proto:
                        insts.pop()
                        blk.instructions = insts
            protos[engine] = proto
        nop = copy.deepcopy(protos[engine])
        nop.name = name
        return nop

    for f in nc.m.functions:
        for blk in f.blocks:
            changed = False
            new = []
            for inst in blk.instructions:
                si = getattr(inst, "sync_info", None)
                waits = list(si.on_wait) if si is not None and si.on_wait else []
                if len(waits) > max_waits:
                    changed = True
                    extra, keep = waits[:-max_waits], waits[-max_waits:]
                    for k, w in enumerate(extra):
                        nop = make_nop(inst.engine, f"{inst.name}-hw{k}")
                        nop.sync_info = mybir.SyncInfo(on_wait=[w], on_update=[])
                        new.append(nop)
                    inst.sync_info = mybir.SyncInfo(
                        on_wait=keep, on_update=list(si.on_update or [])
                    )
                new.append(inst)
            if changed:
                blk.instructions = new


def _build_program():
    import concourse.bass as bass
    import concourse.mybir as mybir
    import concourse.tile as tile

    f32, bf16, f8e4 = mybir.dt.float32, mybir.dt.bfloat16, mybir.dt.float8e4
    nc = bass.Bass("TRN2", target_bir_lowering=False, debug=False,
                   num_devices=N_CORES)

    # x pre-arranged on host: xs[b, blk, c*4+d, q, n] = x[b, c, blk*16+q*4+d, n]
    # so each iteration's A is ONE contiguous 1MB DMA (341GB/s vs ~223 for
    # 4x256KB), landing as SBUF [128=(c,d), 4096=(q,n)].
    xs_d = nc.dram_tensor("xs", [B_PER, N_BLK, 128, 4, N], bf16,
                          kind="ExternalInput")
    # gso^T re-tiled by OUTPUT chunk: gtb[hc, p, ic*128+j] = G[hc*128+j, ic*128+p]
    # so stage2/3 for output chunk hc depend only on gtb[hc]'s 256KB DMA.
    gt_d = nc.dram_tensor("gtb", [NCH, 128, N], bf16, kind="ExternalInput")
    # fp8 copy of gtb[:, :, 0:256] (contraction chunks 0-1): stage2 does
    # those two chunks as ONE DoubleRow matmul at 2x rate.  Error budget:
    # simulated rel-max 1.63e-2 vs the 2e-2 gate (bf16 baseline 4.27e-3).
    g8_d = nc.dram_tensor("g8", [NCH, 128, 2, 128], f8e4, kind="ExternalInput")
    wblk_d = nc.dram_tensor("wblk", [3, 128, 128], bf16, kind="ExternalInput")
    bias_d = nc.dram_tensor("biast", [128, 512], f32, kind="ExternalInput")
    # Stores are contiguous [128, 512] blocks (1KB runs per partition, 128
    # packets) instead of scattered [t, n, j] writes (64B runs, 2048 packets);
    # the host untangles (b, blk, hc, n, t, j) -> (b, t, n_global, j).
    out_d = nc.dram_tensor("out", [B_PER, N_BLK, NCH, 128, 512], bf16,
                           kind="ExternalOutput")

    with tile.TileContext(nc) as tc:
        with (
            tc.tile_pool(name="const", bufs=1) as cpool,
            tc.tile_pool(name="warm", bufs=1) as wpool,
            tc.tile_pool(name="gt", bufs=1) as gtpool,
            tc.tile_pool(name="a", bufs=2) as apool,
            tc.tile_pool(name="p2g", bufs=16) as p2pool,
            tc.tile_pool(name="p28", bufs=2) as p28pool,
            tc.tile_pool(name="ug", bufs=12) as ugpool,
            tc.tile_pool(name="o", bufs=2) as opool,
            tc.tile_pool(name="ps1", bufs=2, space="PSUM") as ps1,
            tc.tile_pool(name="ps2", bufs=3, space="PSUM") as ps2,
            tc.tile_pool(name="ps3", bufs=3, space="PSUM") as ps3,
        ):
            # --- startup ---  First DMA byte flows ~2us after the ~6.7us
            # fixed preamble; total startup bytes ~3.1MB are HBM-bound.
            # Everything rides ONE queue (sync) in priority order so each
            # transfer gets the full ring rate: w22+w1 -> A(iter0, 1MB
            # contiguous) -> gtb col-blocks (stage2[hc] streams as its
            # block lands) -> w02 -> bias.  Meanwhile the PE runs dummy
            # matmuls on a zeroed scratch tile: together with stage1 this
            # gives a ~8us gap-free PE stream, guaranteeing the HAM
            # clock-gate opens (K=8/8) during startup instead of ~15us in.
            scratch = wpool.tile([128, 512], bf16)
            nc.vector.memset(scratch[:], 0.0)

            wtiles = [cpool.tile([128, 128], bf16, tag=f"w{k}", name=f"w{k}")
                      for k in range(3)]
            w1_sb, w22_sb, w02_sb = (t[:] for t in wtiles)
            bias_sb = cpool.tile([128, 512], f32)

            def load_A(b, blk, eng):
                a = apool.tile([128, 4 * N], bf16, tag="a")
                eng.dma_start(a[:], xs_d.ap()[b, blk])
                return a

            g8_sb = gtpool.tile([128, NCH, 2, 128], f8e4, tag="g8", name="g8")

            def load_w12():
                # tiny; ride the otherwise-idle scalar ring so they don't
                # delay A0/gtb on sync (each transfer has ~0.6us fixed cost)
                nc.scalar.dma_start(wtiles[1][:], wblk_d.ap()[1])  # w22
                nc.scalar.dma_start(wtiles[0][:], wblk_d.ap()[0])  # w1
                nc.scalar.dma_start(g8_sb[:], g8_d.ap())

            def load_gt():
                gt_sb = []
                for hc in range(NCH):
                    g = gtpool.tile([128, N], bf16, tag=f"gt{hc}")
                    nc.sync.dma_start(g[:], gt_d.ap()[hc])
                    gt_sb.append(g)
                # w02/bias aren't needed until the first stage3 (~29us in)
                nc.sync.dma_start(wtiles[2][:], wblk_d.ap()[2])
                nc.sync.dma_start(bias_sb[:], bias_d.ap())
                return gt_sb

            def warmup(n_mm=11):
                # ~4.7us of junk matmuls at the cold clock fills the PE
                # until A lands (~11.7us); that contiguous stream contains
                # a full free-running 4096-cycle HAM window, so K=8/8 fires
                # ~10.5us and every real matmul runs at 2.4GHz.  A sub-
                # window idle at the seam cannot re-throttle.
                for _ in range(n_mm):
                    ps = ps1.tile([128, 512], f32, tag="p2")
                    nc.tensor.matmul(ps[:], scratch[:, 0:128], scratch[:],
                                     start=True, stop=True)

            def stage1(A):
                """P2 = x0 @ (2 W2), laid out [n-chunk, (q,d,j)].  Chunks
                0-1 are cast to fp8 (stacked in p28) for stage2's DoubleRow
                matmul; chunks 2-7 to bf16 as before."""
                p28 = p28pool.tile([128, 2, 512], f8e4, tag="p28",
                                   name="p28")
                p2g = []
                for nch in range(NCH):
                    ps = ps1.tile([128, 512], f32, tag="p2")
                    for q in range(4):
                        nc.tensor.matmul(
                            ps[:, q * 128:(q + 1) * 128],
                            A[:, q * N + nch * 128:q * N + (nch + 1) * 128],
                            w22_sb,
                            start=True, stop=True,
                        )
                    if nch < 2:
                        nc.vector.tensor_copy(p28[:, nch], ps[:])
                        p2g.append(None)
                    else:
                        t = p2pool.tile([128, 512], bf16, tag="p2g")
                        nc.vector.tensor_copy(t[:], ps[:])
                        p2g.append(t)
                return p2g, p28

            DR = mybir.MatmulPerfMode.DoubleRow

            def stage2(A, p2g, p28):
                """U = G @ P2 + x0 @ W1.  Contraction chunks 0-1 run as one
                fp8 DoubleRow matmul (256-deep at 2 MACs/cell/cycle)."""
                ug = []
                for nch in range(NCH):
                    ps = ps2.tile([128, 512], f32, tag="u")
                    nc.tensor.matmul(
                        ps[:], g8_sb[:, nch], p28[:],
                        start=True, stop=False, perf_mode=DR,
                    )
                    for ic in range(2, NCH):
                        nc.tensor.matmul(
                            ps[:],
                            gt_sb[nch][:, ic * 128:(ic + 1) * 128],
                            p2g[ic][:],
                            start=False, stop=False,
                        )
                    for q in range(4):
                        nc.tensor.matmul(
                            ps[:, q * 128:(q + 1) * 128],
                            A[:, q * N + nch * 128:q * N + (nch + 1) * 128],
                            w1_sb,
                            start=False, stop=(q == 3),
                        )
                    t = ugpool.tile([128, 512], bf16, tag="ug")
                    nc.vector.tensor_copy(t[:], ps[:])
                    ug.append(t)
                return ug

            def stage3(A, ug, b, blk, last=False):
                """out = G @ U + x0 @ (W0 - W2) + bias, then store."""
                t0 = blk * 16
                o_sb = opool.tile([128, NCH * 512], bf16, tag="o")
                for hc in range(NCH):
                    ps = ps3.tile([128, 512], f32, tag="ou")
                    for ic in range(NCH):
                        nc.tensor.matmul(
                            ps[:],
                            gt_sb[hc][:, ic * 128:(ic + 1) * 128],
                            ug[ic][:],
                            start=(ic == 0), stop=False,
                        )
                    for q in range(4):
                        nc.tensor.matmul(
                            ps[:, q * 128:(q + 1) * 128],
                            A[:, q * N + hc * 128:q * N + (hc + 1) * 128],
                            w02_sb,
                            start=False, stop=(q == 3),
                        )
                    # final iteration: split stores across both queues, and
                    # halve the last two chunks' add+store so each queue's
                    # final transfer is issued ~0.4us earlier (the kernel-end
                    # barrier waits on every store's HBM receipt).
                    eng = nc.scalar if (last and hc % 2) else nc.sync
                    if last and hc >= 6:
                        for h in (0, 1):
                            cs = hc * 512 + h * 256
                            nc.vector.tensor_add(
                                o_sb[:, cs:cs + 256], ps[:, h * 256:(h + 1) * 256],
                                bias_sb[:, h * 256:(h + 1) * 256],
                            )
                            eng.dma_start(
                                out_d.ap()[b, blk, hc, :, h * 256:(h + 1) * 256],
                                o_sb[:, cs:cs + 256],
                            )
                    else:
                        nc.vector.tensor_add(
                            o_sb[:, hc * 512:(hc + 1) * 512], ps[:], bias_sb[:]
                        )
                        eng.dma_start(
                            out_d.ap()[b, blk, hc],
                            o_sb[:, hc * 512:(hc + 1) * 512],
                        )

            # --- software-pipelined main loop over 8 (b, blk) iterations ---
            # All startup transfers ride the sync ring in strict priority
            # order (a second active ring would steal packet slots from
            # this one): A0, w22, w1, gtb blocks, w02, bias, then A1.
            # Steady-state prefetch moves to scalar; stores ride sync.
            iters = [(b, blk) for b in range(B_PER) for blk in range(N_BLK)]
            A_cur = load_A(*iters[0], eng=nc.sync)
            load_w12()
            gt_sb = load_gt()
            warmup()
            p2g_cur, p28_cur = stage1(A_cur)
            for k in range(len(iters)):
                ug = stage2(A_cur, p2g_cur, p28_cur)
                if k + 1 < len(iters):
                    A_nxt = load_A(*iters[k + 1],
                                   eng=nc.sync if k == 0 else nc.scalar)
                    p2g_nxt, p28_nxt = stage1(A_nxt)
                else:
                    A_nxt = p2g_nxt = p28_nxt = None
                stage3(A_cur, ug, *iters[k], last=(k + 1 == len(iters)))
                A_cur, p2g_cur, p28_cur = A_nxt, p2g_nxt, p28_nxt

    _split_multi_waits(nc, mybir)
    return nc


def _prep_inputs(x, gso, weight, bias):
    """Host-side shard + repack.  Returns per-core in_maps."""
    import ml_dtypes
    bf16 = ml_dtypes.bfloat16
    w1, w22, w02 = weight[1], 2.0 * weight[2], weight[0] - weight[2]
    rows = (np.arange(C_IN)[None, :] * 4).repeat(4, 0) + np.arange(4)[:, None]
    wblk = np.zeros((3, 128, 128), np.float32)
    for k, w in enumerate((w1, w22, w02)):
        for d in range(4):
            wblk[k, rows[d], d * 32:(d + 1) * 32] = w
    wblk = wblk.astype(bf16)
    # gtb[hc, p, ic, col] = G^T[ic*128+p, hc*128+col]: stage2/3 output
    # chunk hc reads only gtb[hc] (one contiguous 256KB block).
    gt = np.ascontiguousarray(gso.T)
    gtbf = np.ascontiguousarray(
        gt.reshape(NCH, 128, NCH, 128).transpose(2, 1, 0, 3)
    ).reshape(NCH, 128, N)
    gtb = gtbf.astype(bf16)
    # fp8 stationary for stage2's DoubleRow matmul over contraction
    # chunks 0-1: g8[p, hc, ko, m] = G^T[ko*128+p, hc*128+m] in e4m3
    f8 = ml_dtypes.float8_e4m3fn
    g8 = np.ascontiguousarray(
        gtbf[:, :, 0:256].transpose(1, 0, 2)
    ).reshape(128, NCH, 2, 128).astype(f8)
    biast = np.tile(np.asarray(bias, np.float32), (128, 16))
    # xs[b, blk, c*4+d, q, n] = x[b, c, blk*16 + q*4 + d, n]: each (b, blk)
    # iteration's A-tile is one contiguous 1MB DMA.
    xr = (x.astype(bf16)
          .reshape(B, C_IN, N_BLK, 4, 4, N)    # t -> (blk, q, d)
          .transpose(0, 2, 1, 4, 3, 5)         # -> [b, blk, c, d, q, n]
          .reshape(B, N_BLK, 128, 4, N))
    in_maps = []
    for c in range(N_CORES):
        in_maps.append({
            "xs": np.ascontiguousarray(xr[c * B_PER:(c + 1) * B_PER]),
            "gtb": gtb,
            "g8": g8,
            "wblk": wblk,
            "biast": biast,
        })
    return in_maps


def kernel(x, gso, weight, bias):
    from concourse import bass_utils

    x = np.asarray(x, np.float32)
    gso = np.asarray(gso, np.float32)
    weight = np.asarray(weight, np.float32)
    bias = np.asarray(bias, np.float32)

    if "nc" not in _CACHE:
        _CACHE["nc"] = _build_program()
    nc = _CACHE["nc"]

    in_maps = _prep_inputs(x, gso, weight, bias)
    res = bass_utils.run_bass_kernel_spmd(
        nc, in_maps, core_ids=list(range(N_CORES))
    )
    # [b, blk, hc, n128, (t16, j32)] -> [B, T, N, C_OUT]
    out = np.concatenate([r["out"] for r in res.results], axis=0)
    out = out.reshape(B, N_BLK, NCH, 128, 16, C_OUT)
    out = out.transpose(0, 1, 4, 2, 3, 5).reshape(B, T, N, C_OUT)
    return np.ascontiguousarray(out).astype(np.float32)



# revision 35
# speedup vs baseline: 1.1891x; 1.1891x over previous
"""ChebGraphConv (K=3) Trainium2 kernel.

Reference computation (per batch b, time t; x0 = x[b,:,t,:].T in [N, C_IN]):
    out = x0 @ W0 + (G @ x0) @ W1 + (2 G (G @ x0) - x0) @ W2 + bias
Rewritten (G commutes with channel matmuls):
    P2  = x0 @ (2 W2)
    U   = G @ P2 + x0 @ W1
    out = G @ U  + x0 @ (W0 - W2) + bias

Sharding: batch B=16 split over 8 cores (2 b per core).  gso/weights
replicated.  All matmuls in bfloat16 (full PE rate at any output width;
fp32r moving operands drop to 1/4 rate below 256-wide).  PSUM
accumulation stays fp32; output stored bf16 as contiguous [128,512]
blocks (1KB DMA runs; host untangles the layout and upcasts — rel err
~4.3e-3 vs the 2e-2 gate; fp8/DoubleRow measured at 3-4.5e-2 and
rejected).  Steady state is PE-bound at ~267us busy (bf16 floor 259.5us
= 77824 rows/iter x 8 iters at 2.4GHz); DVE casts/adds and both DMA
queues have 2x+ slack; zero mid-stream PE gaps.  Measured: 287763 ns.

Per core loop over 8 blocks (b, blk) with 16 t's each:
  A[q]   = x[b, :, blk*16+4q : +4, :]  as [128=(c,d4), 1024=n]   (4 quads)
  S1: P2[nchunk 128, 512=(q,d,j)] = sum_c A[q]^T Wblk(2W2)        (block-diag)
  S2: U = G-matmuls (lhsT = gsoT chunks) + block-diag W1 accum
  S3: out = G-matmuls on U + block-diag (W0-W2) accum, + bias, DMA out
"""
import numpy as np

B, C_IN, T, N = 16, 32, 64, 1024
C_OUT = 32
N_CORES = 8
B_PER = B // N_CORES          # 2
N_BLK = T // 16               # 4 blocks of 16 t's per b -> 8 iters per core
NCH = N // 128                # 8 chunks of the node dim

_CACHE = {}


def _split_multi_waits(nc, mybir, max_waits: int = 1):
    """Walrus rejects instructions whose ISA struct can't hold all their sync
    waits (fp32 self-loading matmul: 1).  Hoist excess waits onto inserted
    same-engine NoOps, which execute in order before the instruction."""
    import copy

    protos = {}

    def make_nop(engine, name):
        if engine not in protos:
            eng_map = {
                mybir.EngineType.SP: nc.sync,
                mybir.EngineType.PE: nc.tensor,
                mybir.EngineType.DVE: nc.vector,
                mybir.EngineType.Activation: nc.scalar,
                mybir.EngineType.Pool: nc.gpsimd,
            }
            proto = eng_map[engine].nop().ins
            for f in nc.m.functions:
                for blk in f.blocks:
                    insts = list(blk.instructions)
                    if insts and insts[-1] is proto:
                        insts.pop()
                        blk.instructions = insts
            protos[engine] = proto
        nop = copy.deepcopy(protos[engine])
        nop.name = name
        return nop

    for f in nc.m.functions:
        for blk in f.blocks:
            changed = False
            new = []
            for inst in blk.instructions:
                si = getattr(inst, "sync_info", None)
                waits = list(si.on_wait) if si is not None and si.on_wait else []
                if len(waits) > max_waits:
                    changed = True
                    extra, keep = waits[:-max_waits], waits[-max_waits:]
                    for k, w in enumerate(extra):
                        nop = make_nop(inst.engine, f"{inst.name}-hw{k}")
                        nop.sync_info = mybir.SyncInfo(on_wait=[w], on_update=[])
                        new.append(nop)
                    inst.sync_info = mybir.SyncInfo(
                        on_wait=keep, on_update=list(si.on_update or [])
                    )
                new.append(inst)
            if changed:
                blk.instructions = new


def _build_program():
    import concourse.bass as bass
    import concourse.mybir as mybir
    import concourse.tile as tile

    f32, bf16, f8e4 = mybir.dt.float32, mybir.dt.bfloat16, mybir.dt.float8e4
    nc = bass.Bass("TRN2", target_bir_lowering=False, debug=False,
                   num_devices=N_CORES)

    # x pre-arranged on host: xs[b, blk, c*4+d, q, n] = x[b, c, blk*16+q*4+d, n]
    # so each iteration's A is ONE contiguous 1MB DMA (341GB/s vs ~223 for
    # 4x256KB), landing as SBUF [128=(c,d), 4096=(q,n)].
    xs_d = nc.dram_tensor("xs", [B_PER, N_BLK, 128, 4, N], bf16,
                          kind="ExternalInput")
    # gso^T re-tiled by OUTPUT chunk: gtb[hc, p, ic*128+j] = G[hc*128+j, ic*128+p]
    # so stage2/3 for output chunk hc depend only on gtb[hc]'s 256KB DMA.
    gt_d = nc.dram_tensor("gtb", [NCH, 128, N], bf16, kind="ExternalInput")
    # fp8 copy of gtb[:, :, 0:256] (contraction chunks 0-1): stage2 does
    # those two chunks as ONE DoubleRow matmul at 2x rate.  Error budget:
    # simulated rel-max 1.63e-2 vs the 2e-2 gate (bf16 baseline 4.27e-3).
    g8_d = nc.dram_tensor("g8", [NCH, 128, 2, 128], f8e4, kind="ExternalInput")
    wblk_d = nc.dram_tensor("wblk", [3, 128, 128], bf16, kind="ExternalInput")
    bias_d = nc.dram_tensor("biast", [128, 512], f32, kind="ExternalInput")
    # Stores are contiguous [128, 512] blocks (1KB runs per partition, 128
    # packets) instead of scattered [t, n, j] writes (64B runs, 2048 packets);
    # the host untangles (b, blk, hc, n, t, j) -> (b, t, n_global, j).
    out_d = nc.dram_tensor("out", [B_PER, N_BLK, NCH, 128, 512], bf16,
                           kind="ExternalOutput")

    with tile.TileContext(nc) as tc:
        with (
            tc.tile_pool(name="const", bufs=1) as cpool,
            tc.tile_pool(name="warm", bufs=1) as wpool,
            tc.tile_pool(name="gt", bufs=1) as gtpool,
            tc.tile_pool(name="a", bufs=2) as apool,
            tc.tile_pool(name="p2g", bufs=16) as p2pool,
            tc.tile_pool(name="p28", bufs=2) as p28pool,
            tc.tile_pool(name="ug", bufs=12) as ugpool,
            tc.tile_pool(name="o", bufs=2) as opool,
            tc.tile_pool(name="ps1", bufs=2, space="PSUM") as ps1,
            tc.tile_pool(name="ps2", bufs=3, space="PSUM") as ps2,
            tc.tile_pool(name="ps3", bufs=3, space="PSUM") as ps3,
        ):
            # --- startup ---  First DMA byte flows ~2us after the ~6.7us
            # fixed preamble; total startup bytes ~3.1MB are HBM-bound.
            # Everything rides ONE queue (sync) in priority order so each
            # transfer gets the full ring rate: w22+w1 -> A(iter0, 1MB
            # contiguous) -> gtb col-blocks (stage2[hc] streams as its
            # block lands) -> w02 -> bias.  Meanwhile the PE runs dummy
            # matmuls on a zeroed scratch tile: together with stage1 this
            # gives a ~8us gap-free PE stream, guaranteeing the HAM
            # clock-gate opens (K=8/8) during startup instead of ~15us in.
            scratch = wpool.tile([128, 512], bf16)
            nc.vector.memset(scratch[:], 0.0)

            wtiles = [cpool.tile([128, 128], bf16, tag=f"w{k}", name=f"w{k}")
                      for k in range(3)]
            w1_sb, w22_sb, w02_sb = (t[:] for t in wtiles)
            bias_sb = cpool.tile([128, 512], f32)

            def load_A(b, blk, eng):
                a = apool.tile([128, 4 * N], bf16, tag="a")
                eng.dma_start(a[:], xs_d.ap()[b, blk])
                return a

            g8_sb = gtpool.tile([128, NCH, 2, 128], f8e4, tag="g8", name="g8")

            def load_w12():
                # tiny; ride the otherwise-idle scalar ring so they don't
                # delay A0/gtb on sync (each transfer has ~0.6us fixed cost)
                nc.scalar.dma_start(wtiles[1][:], wblk_d.ap()[1])  # w22
                nc.scalar.dma_start(wtiles[0][:], wblk_d.ap()[0])  # w1
                nc.scalar.dma_start(g8_sb[:], g8_d.ap())

            def load_gt():
                gt_sb = []
                for hc in range(NCH):
                    g = gtpool.tile([128, N], bf16, tag=f"gt{hc}")
                    nc.sync.dma_start(g[:], gt_d.ap()[hc])
                    gt_sb.append(g)
                # w02/bias aren't needed until the first stage3 (~29us in)
                nc.sync.dma_start(wtiles[2][:], wblk_d.ap()[2])
                nc.sync.dma_start(bias_sb[:], bias_d.ap())
                return gt_sb

            def warmup(n_mm=11):
                # ~4.7us of junk matmuls at the cold clock fills the PE
                # until A lands (~11.7us); that contiguous stream contains
                # a full free-running 4096-cycle HAM window, so K=8/8 fires
                # ~10.5us and every real matmul runs at 2.4GHz.  A sub-
                # window idle at the seam cannot re-throttle.
                for _ in range(n_mm):
                    ps = ps1.tile([128, 512], f32, tag="p2")
                    nc.tensor.matmul(ps[:], scratch[:, 0:128], scratch[:],
                                     start=True, stop=True)

            def stage1(A):
                """P2 = x0 @ (2 W2), laid out [n-chunk, (q,d,j)].  Chunks
                0-1 are cast to fp8 (stacked in p28) for stage2's DoubleRow
                matmul; chunks 2-7 to bf16 as before."""
                p28 = p28pool.tile([128, 2, 512], f8e4, tag="p28",
                                   name="p28")
                p2g = []
                for nch in range(NCH):
                    ps = ps1.tile([128, 512], f32, tag="p2")
                    for q in range(4):
                        nc.tensor.matmul(
                            ps[:, q * 128:(q + 1) * 128],
                            A[:, q * N + nch * 128:q * N + (nch + 1) * 128],
                            w22_sb,
                            start=True, stop=True,
                        )
                    if nch < 2:
                        nc.vector.tensor_copy(p28[:, nch], ps[:])
                        p2g.append(None)
                    else:
                        t = p2pool.tile([128, 512], bf16, tag="p2g")
                        nc.vector.tensor_copy(t[:], ps[:])
                        p2g.append(t)
                return p2g, p28

            DR = mybir.MatmulPerfMode.DoubleRow

            def stage2(A, p2g, p28):
                """U = G @ P2 + x0 @ W1.  Contraction chunks 0-1 run as one
                fp8 DoubleRow matmul (256-deep at 2 MACs/cell/cycle)."""
                ug = []
                for nch in range(NCH):
                    ps = ps2.tile([128, 512], f32, tag="u")
                    nc.tensor.matmul(
                        ps[:], g8_sb[:, nch], p28[:],
                        start=True, stop=False, perf_mode=DR,
                    )
                    for ic in range(2, NCH):
                        nc.tensor.matmul(
                            ps[:],
                            gt_sb[nch][:, ic * 128:(ic + 1) * 128],
                            p2g[ic][:],
                            start=False, stop=False,
                        )
                    for q in range(4):
                        nc.tensor.matmul(
                            ps[:, q * 128:(q + 1) * 128],
                            A[:, q * N + nch * 128:q * N + (nch + 1) * 128],
                            w1_sb,
                            start=False, stop=(q == 3),
                        )
                    t = ugpool.tile([128, 512], bf16, tag="ug")
                    nc.vector.tensor_copy(t[:], ps[:])
                    ug.append(t)
                return ug

            def stage3(A, ug, b, blk, last=False):
                """out = G @ U + x0 @ (W0 - W2) + bias, then store."""
                t0 = blk * 16
                o_sb = opool.tile([128, NCH * 512], bf16, tag="o")
                for hc in range(NCH):
                    ps = ps3.tile([128, 512], f32, tag="ou")
                    for ic in range(NCH):
                        nc.tensor.matmul(
                            ps[:],
                            gt_sb[hc][:, ic * 128:(ic + 1) * 128],
                            ug[ic][:],
                            start=(ic == 0), stop=False,
                        )
                    for q in range(4):
                        nc.tensor.matmul(
                            ps[:, q * 128:(q + 1) * 128],
                            A[:, q * N + hc * 128:q * N + (hc + 1) * 128],
                            w02_sb,
                            start=False, stop=(q == 3),
                        )
                    nc.vector.tensor_add(
                        o_sb[:, hc * 512:(hc + 1) * 512], ps[:], bias_sb[:]
                    )
                    # final iteration: split stores across both queues to
                    # halve the end-of-kernel drain (scalar queue is idle)
                    eng = nc.scalar if (last and hc % 2) else nc.sync
                    eng.dma_start(
                        out_d.ap()[b, blk, hc],
                        o_sb[:, hc * 512:(hc + 1) * 512],
                    )

            # --- software-pipelined main loop over 8 (b, blk) iterations ---
            # All startup transfers ride the sync ring in strict priority
            # order (a second active ring would steal packet slots from
            # this one): A0, w22, w1, gtb blocks, w02, bias, then A1.
            # Steady-state prefetch moves to scalar; stores ride sync.
            iters = [(b, blk) for b in range(B_PER) for blk in range(N_BLK)]
            A_cur = load_A(*iters[0], eng=nc.sync)
            load_w12()
            gt_sb = load_gt()
            warmup()
            p2g_cur, p28_cur = stage1(A_cur)
            for k in range(len(iters)):
                ug = stage2(A_cur, p2g_cur, p28_cur)
                if k + 1 < len(iters):
                    A_nxt = load_A(*iters[k + 1],
                                   eng=nc.sync if k == 0 else nc.scalar)
                    p2g_nxt, p28_nxt = stage1(A_nxt)
                else:
                    A_nxt = p2g_nxt = p28_nxt = None
                stage3(A_cur, ug, *iters[k], last=(k + 1 == len(iters)))
                A_cur, p2g_cur, p28_cur = A_nxt, p2g_nxt, p28_nxt

    _split_multi_waits(nc, mybir)
    return nc


def _prep_inputs(x, gso, weight, bias):
    """Host-side shard + repack.  Returns per-core in_maps."""
    import ml_dtypes
    bf16 = ml_dtypes.bfloat16
    w1, w22, w02 = weight[1], 2.0 * weight[2], weight[0] - weight[2]
    rows = (np.arange(C_IN)[None, :] * 4).repeat(4, 0) + np.arange(4)[:, None]
    wblk = np.zeros((3, 128, 128), np.float32)
    for k, w in enumerate((w1, w22, w02)):
        for d in range(4):
            wblk[k, rows[d], d * 32:(d + 1) * 32] = w
    wblk = wblk.astype(bf16)
    # gtb[hc, p, ic, col] = G^T[ic*128+p, hc*128+col]: stage2/3 output
    # chunk hc reads only gtb[hc] (one contiguous 256KB block).
    gt = np.ascontiguousarray(gso.T)
    gtbf = np.ascontiguousarray(
        gt.reshape(NCH, 128, NCH, 128).transpose(2, 1, 0, 3)
    ).reshape(NCH, 128, N)
    gtb = gtbf.astype(bf16)
    # fp8 stationary for stage2's DoubleRow matmul over contraction
    # chunks 0-1: g8[p, hc, ko, m] = G^T[ko*128+p, hc*128+m] in e4m3
    f8 = ml_dtypes.float8_e4m3fn
    g8 = np.ascontiguousarray(
        gtbf[:, :, 0:256].transpose(1, 0, 2)
    ).reshape(128, NCH, 2, 128).astype(f8)
    biast = np.tile(np.asarray(bias, np.float32), (128, 16))
    # xs[b, blk, c*4+d, q, n] = x[b, c, blk*16 + q*4 + d, n]: each (b, blk)
    # iteration's A-tile is one contiguous 1MB DMA.
    xr = (x.astype(bf16)
          .reshape(B, C_IN, N_BLK, 4, 4, N)    # t -> (blk, q, d)
          .transpose(0, 2, 1, 4, 3, 5)         # -> [b, blk, c, d, q, n]
          .reshape(B, N_BLK, 128, 4, N))
    in_maps = []
    for c in range(N_CORES):
        in_maps.append({
            "xs": np.ascontiguousarray(xr[c * B_PER:(c + 1) * B_PER]),
            "gtb": gtb,
            "g8": g8,
            "wblk": wblk,
            "biast": biast,
        })
    return in_maps


def kernel(x, gso, weight, bias):
    from concourse import bass_utils

    x = np.asarray(x, np.float32)
    gso = np.asarray(gso, np.float32)
    weight = np.asarray(weight, np.float32)
    bias = np.asarray(bias, np.float32)

    if "nc" not in _CACHE:
        _CACHE["nc"] = _build_program()
    nc = _CACHE["nc"]

    in_maps = _prep_inputs(x, gso, weight, bias)
    res = bass_utils.run_bass_kernel_spmd(
        nc, in_maps, core_ids=list(range(N_CORES))
    )
    # [b, blk, hc, n128, (t16, j32)] -> [B, T, N, C_OUT]
    out = np.concatenate([r["out"] for r in res.results], axis=0)
    out = out.reshape(B, N_BLK, NCH, 128, 16, C_OUT)
    out = out.transpose(0, 1, 4, 2, 3, 5).reshape(B, T, N, C_OUT)
    return np.ascontiguousarray(out).astype(np.float32)

